# revision 14
# baseline (speedup 1.0000x reference)
"""CompartmentAwareNormalization Trainium2 kernel (v2: sorted-compartment).

Math (reference, per token t with d_model D=1024, NC=5 compartments):
    mu, var = stats(x_t) (biased, over D)
    normed  = (x_t - mu) * rsqrt(var + eps)
    y_t     = (normed * gamma[c] + beta[c]) * scale[c],   c = cid_t
    out_t   = y_t + y_t @ W.T + b = y_t @ W2 + b,         W2 = W.T + I

Rewrite: with Gp = gamma*scale, Bp = beta*scale, W2c[c] = diag(Gp[c]) @ W2,
GW = Gp @ W2, A = Bp @ W2 + b:
    out_t = istd_t * (x_t @ W2c[c]) + (-mu_t)*istd_t*GW[c] + A[c]
          = istd_t * ( x_t @ W2c[c] + (-mu_t)*GW[c] + std_t*A[c] )

Host-side: tokens are sorted by compartment and dealt to the 8 cores so that
every core holds exactly Lc[c] tokens of compartment c (identical layout on
all cores -> one SPMD program).  Compartment-count remainders (< 8 tokens per
compartment) and any out-of-range ids are computed on the host in numpy.
x is sent as fp16 in a block-contiguous d-major layout; output returns fp16.

On device, per 512-token block:
    s1 = ones128 @ x        -> [128, TB] (replicated over partitions)
    s2 = ones128 @ x^2
    istd/mu/std derived on DVE/ACT, replicated on all partitions
    out_psum[j] = sum_k W2c[c][kchunk].T @ x[kchunk]   (fp16 matmuls)
                + [GW[c]; A[c]].T @ [negmu; std]       (K=2 aux matmul)
    out = out_psum * istd   (DVE multiply = PSUM evacuation)
"""

import numpy as np
from contextlib import ExitStack

import concourse.bass as bass
import concourse.bacc as bacc
import concourse.tile as tile
from concourse import mybir
from concourse import bass_utils

B, S, D = 4, 8192, 1024
NC = 5
EPS = 1e-5
NCORES = 8
T = B * S                  # 32768 tokens total
TPC = T // NCORES          # 4096 tokens per core
TB = 512                   # tokens per device block
NBLK = TPC // TB           # 8
NKC = D // 128             # contraction chunks
NET = D // 128             # output e-tiles
F32 = mybir.dt.float32
F16 = mybir.dt.float16


# --------------------------------------------------------------------------
# host-side planning


def _plan(cid):
    """cid: int array [T]. Returns (segs, core_tokens, host_idx) where
    segs = tuple over blocks of tuples of (c, t0, n) segments (same for all
    cores), core_tokens = int32 [NCORES, TPC] global token index per device
    slot (-1 = dummy), host_idx = tokens to compute on the host."""
    valid = (cid >= 0) & (cid < NC)
    idx_by_c = [np.nonzero(valid & (cid == c))[0] for c in range(NC)]
    counts = np.array([len(ix) for ix in idx_by_c])
    Lc = counts // NCORES
    rem = counts - Lc * NCORES

    host_idx = np.concatenate(
        [ix[: rem[c]] for c, ix in enumerate(idx_by_c)]
        + [np.nonzero(~valid)[0]]
    ).astype(np.int64)

    # per-core run lengths, pad last compartment with dummies up to TPC
    Lc2 = Lc.copy()
    pad = TPC - int(Lc.sum())
    assert pad >= 0
    Lc2[NC - 1] += pad

    # runs in order c=0..NC-1, run c occupying Lc2[c] slots (dummies = -1)
    core_tokens = np.full((NCORES, TPC), -1, dtype=np.int64)
    for r in range(NCORES):
        pos = 0
        for c in range(NC):
            n = int(Lc[c])
            seg = idx_by_c[c][rem[c] + r * n : rem[c] + (r + 1) * n]
            core_tokens[r, pos : pos + n] = seg
            pos += int(Lc2[c])

    # segments per block
    bounds = np.cumsum(Lc2)        # run end positions within a core
    segs = []
    for b in range(NBLK):
        t0, t1 = b * TB, (b + 1) * TB
        bsegs = []
        lo = t0
        for c in range(NC):
            run_lo = 0 if c == 0 else int(bounds[c - 1])
            run_hi = int(bounds[c])
            a, z = max(lo, run_lo), min(t1, run_hi)
            if z > a:
                bsegs.append((c, a - t0, z - a))
        segs.append(tuple(bsegs))
    return tuple(segs), core_tokens, host_idx


def _prep_host(x, compartment_ids, gamma, beta, scale, W, b):
    x2d = np.asarray(x, dtype=np.float32).reshape(T, D)
    cid = np.asarray(compartment_ids).reshape(T).astype(np.int64)
    gamma = np.asarray(gamma, dtype=np.float32)
    beta = np.asarray(beta, dtype=np.float32)
    scale = np.asarray(scale, dtype=np.float32)
    W = np.asarray(W, dtype=np.float32)
    b = np.asarray(b, dtype=np.float32)

    segs, core_tokens, host_idx = _plan(cid)

    Gp = gamma * scale[:, None]
    Bp = beta * scale[:, None]
    W2 = W.T.astype(np.float32) + np.eye(D, dtype=np.float32)
    GW = (Gp @ W2).astype(np.float16)
    A = (Bp @ W2 + b).astype(np.float16)
    W2_16 = np.ascontiguousarray(W2).astype(np.float16)

    # gpt[p, k*NC + c] = Gp[c, 128k + p]
    gpt = np.ascontiguousarray(
        Gp.reshape(NC, NKC, 128).transpose(2, 1, 0).reshape(128, NKC * NC)
    ).astype(np.float32)
    # gwa[32r + 0, c*D + e] = GW[c, e]; gwa[32r + 1, c*D + e] = A[c, e]
    # (replicated at the 4 row-group base partitions for tile_position packing)
    gwa = np.zeros((128, NC * D), dtype=np.float16)
    for r in range(4):
        gwa[32 * r + 0] = GW.reshape(-1)
        gwa[32 * r + 1] = A.reshape(-1)

    # x packed per core: [NBLK, 128, NKC, TB] fp16,
    # xpk[b, p, k, t] = x2d[token(512b + t), 128k + p]
    in_maps = []
    for r in range(NCORES):
        toks = core_tokens[r]
        xs = x2d[np.clip(toks, 0, T - 1)].astype(np.float16)
        xs[toks < 0] = 0
        a4 = xs.reshape(NBLK, TB, NKC, 128).transpose(0, 3, 2, 1)
        in_maps.append({
            "x": np.ascontiguousarray(a4),
            "w2": W2_16,
            "gpt": gpt,
            "gwa": gwa,
        })

    aux = dict(
        segs=segs, core_tokens=core_tokens, host_idx=host_idx,
        x2d=x2d, cid=cid, gamma=gamma, beta=beta, scale=scale, W=W, b=b,
    )
    return in_maps, aux


def _host_fixup(aux, out2d):
    """Compute the host-assigned tokens exactly (fp32 numpy)."""
    hi = aux["host_idx"]
    if len(hi) == 0:
        return
    x = aux["x2d"][hi]                       # [n, D]
    cid = aux["cid"][hi]
    valid = (cid >= 0) & (cid < NC)
    c = np.clip(cid, 0, NC - 1)
    mu = x.mean(axis=1, keepdims=True)
    var = np.square(x - mu).mean(axis=1, keepdims=True)
    normed = (x - mu) / np.sqrt(var + EPS)
    y = (normed * aux["gamma"][c] + aux["beta"][c]) * aux["scale"][c][:, None]
    y = np.where(valid[:, None], y, x)
    out2d[hi] = y + y @ aux["W"].T + aux["b"]


def _assemble(results, aux):
    out2d = np.zeros((T, D), dtype=np.float32)
    for r in range(NCORES):
        o = results[r]["out"]                # [NBLK, 128, NET, TB] fp16
        oc = o.transpose(0, 3, 2, 1).reshape(TPC, D).astype(np.float32)
        toks = aux["core_tokens"][r]
        m = toks >= 0
        out2d[toks[m]] = oc[m]
    _host_fixup(aux, out2d)
    return out2d.reshape(B, S, D)


# --------------------------------------------------------------------------
# numpy emulation of the device program (for validating the plan/packing)


def _emulate_core(in_map, segs):
    xt = in_map["x"]                          # [NBLK, 128, NKC, TB] f16
    w2 = in_map["w2"].astype(np.float32)      # [D, D]
    gpt = in_map["gpt"].astype(np.float32)    # [128, NKC*NC]
    gwa = in_map["gwa"].astype(np.float32)    # [128, NC*D]; rows 0/1 = GW/A
    gpt = gpt.reshape(128, NKC, NC)
    w2r = w2.reshape(NKC, 128, D)             # [k, p, e]
    w2c = np.empty((NC, NKC, 128, D), dtype=np.float16)
    for c in range(NC):
        for k in range(NKC):
            w2c[c, k] = (w2r[k].astype(np.float16).astype(np.float32)
                         * gpt[:, k, c : c + 1]).astype(np.float16)
    out = np.zeros((NBLK, 128, NET, TB), dtype=np.float16)
    for b in range(NBLK):
        xb = xt[b].astype(np.float32)         # [128, NKC, TB]
        s1 = xb.sum(axis=(0, 1))              # [TB]
        s2 = (np.square(xb.astype(np.float16).astype(np.float32))
              .astype(np.float16).astype(np.float32).sum(axis=(0, 1)))
        negmu = -s1 / D
        varD = s2 + s1 * negmu
        istd = 1.0 / np.sqrt(varD / D + EPS)
        std = np.sqrt(varD / D + EPS).astype(np.float16).astype(np.float32)
        nm16 = negmu.astype(np.float16).astype(np.float32)
        op = np.zeros((NET, 128, TB), dtype=np.float32)
        for (c, t0, n) in segs[b]:
            sl = slice(t0, t0 + n)
            for j in range(NET):
                esl = slice(j * 128, (j + 1) * 128)
                acc = np.zeros((128, n), dtype=np.float32)
                for k in range(NKC):
                    acc += (w2c[c, k, :, esl].astype(np.float32).T
                            @ xb[:, k, sl])
                acc += (gwa[0, c * D : (c + 1) * D][esl, None] * nm16[None, sl]
                        + gwa[1, c * D : (c + 1) * D][esl, None] * std[None, sl])
                op[j][:, sl] = acc
        for j in range(NET):
            out[b, :, j, :] = (op[j] * istd[None, :]).astype(np.float16)
    return {"out": out}


# --------------------------------------------------------------------------
# device kernel


def _build_nc(segs, repeat=1):
    nc = bacc.Bacc()
    x = nc.declare_dram_parameter("x", [NBLK, 128, NKC, TB], F16, False)
    w2 = nc.declare_dram_parameter("w2", [D, D], F16, False)
    gpt = nc.declare_dram_parameter("gpt", [128, NKC * NC], F32, False)
    gwa = nc.declare_dram_parameter("gwa", [128, NC * D], F16, False)
    out = nc.declare_dram_parameter("out", [NBLK, 128, NET, TB], F16, True)

    with tile.TileContext(nc) as tc, ExitStack() as ctx:
        singles = ctx.enter_context(tc.tile_pool(name="singles", bufs=1))
        xpool = ctx.enter_context(tc.tile_pool(name="xpool", bufs=3))
        sqpool = ctx.enter_context(tc.tile_pool(name="sqpool", bufs=2))
        statp = ctx.enter_context(tc.tile_pool(name="statp", bufs=2))
        auxp = ctx.enter_context(tc.tile_pool(name="auxp", bufs=2))
        osb = ctx.enter_context(tc.tile_pool(name="osb", bufs=2))
        spsum = ctx.enter_context(tc.tile_pool(name="spsum", bufs=2, space="PSUM"))
        opsum = ctx.enter_context(tc.tile_pool(name="opsum", bufs=4, space="PSUM"))

        # resident tables
        w2sb = singles.tile([128, NKC, D], F16)
        w2_r = w2.rearrange("(k p) e -> p k e", p=128)
        for k in range(NKC):
            nc.sync.dma_start(out=w2sb[:, k, :], in_=w2_r[:, k, :])
        gptsb = singles.tile([128, NKC, NC], F32)
        nc.sync.dma_start(out=gptsb, in_=gpt.rearrange("p (k c) -> p k c", c=NC))
        gwasb = singles.tile([128, NC, D], F16)
        nc.sync.dma_start(out=gwasb, in_=gwa.rearrange("p (c e) -> p c e", e=D))
        ones = singles.tile([128, 128], F16)
        nc.vector.memset(ones, 1.0)
        eps_ap = singles.tile([128, 1], F32)
        nc.vector.memset(eps_ap, EPS)

        # per-compartment scaled weights W2c[c][d, e] = Gp[c, d] * W2[d, e]
        w2c = singles.tile([128, NC, NKC, D], F16)
        for c in range(NC):
            for k in range(NKC):
                nc.vector.tensor_scalar_mul(
                    w2c[:, c, k, :], w2sb[:, k, :], gptsb[:, k, c : c + 1])

        rep_ctx = (
            tc.For_i(0, repeat, 1, hint_engines=(mybir.EngineType.PE,),
                     staggered_reset=True)
            if repeat > 1 else None
        )
        if rep_ctx is not None:
            ctx.enter_context(rep_ctx)

        # software pipeline: block b's stats matmuls are issued one block
        # ahead (before block b-1's main matmuls), so the stats -> DVE ->
        # ACT -> auxrhs-DMA chain is long done when PE reaches block b's
        # aux matmuls.  State carried between stages:
        stage = [None] * NBLK   # per-block dict with xt/istdb/auxrhs tiles

        def emit_stats(b):
            xt = xpool.tile([128, NKC, TB], F16)
            nc.sync.dma_start(out=xt, in_=x[b])
            xsq = sqpool.tile([128, NKC, TB], F16)
            nc.scalar.square(out=xsq, in_=xt)

            s1 = spsum.tile([128, TB], F32)
            s2 = spsum.tile([128, TB], F32)
            for k in range(NKC):
                nc.tensor.matmul(s1, ones, xt[:, k, :],
                                 start=(k == 0), stop=(k == NKC - 1))
            for k in range(NKC):
                nc.tensor.matmul(s2, ones, xsq[:, k, :],
                                 start=(k == 0), stop=(k == NKC - 1))

            negmu16 = auxp.tile([4, TB], F16)
            nc.scalar.activation(
                out=negmu16, in_=s1[0:4, :],
                func=mybir.ActivationFunctionType.Copy, scale=-1.0 / D)
            t1 = statp.tile([128, TB], F32)                # D*mu^2
            nc.scalar.activation(
                out=t1, in_=s1,
                func=mybir.ActivationFunctionType.Square, scale=float(D) ** -0.5)
            varD = statp.tile([128, TB], F32)
            nc.vector.tensor_sub(varD, s2, t1)             # D*var
            istdb = statp.tile([128, TB], F32)
            nc.scalar.activation(
                out=istdb, in_=varD,
                func=mybir.ActivationFunctionType.Abs_reciprocal_sqrt,
                bias=eps_ap[:, :], scale=1.0 / D)
            std16 = auxp.tile([4, TB], F16)
            nc.scalar.activation(
                out=std16, in_=varD[0:4, :],
                func=mybir.ActivationFunctionType.Sqrt,
                bias=eps_ap[0:4, :], scale=1.0 / D)

            # auxrhs[32r + 0] = -mu, auxrhs[32r + 1] = std  (4 row groups)
            auxrhs = auxp.tile([128, TB], F16)
            ar = auxrhs.rearrange("(r q) t -> r q t", q=32)
            nc.sync.dma_start(out=ar[:, 0, :], in_=negmu16)
            nc.sync.dma_start(out=ar[:, 1, :], in_=std16)
            stage[b] = dict(xt=xt, istdb=istdb, auxrhs=auxrhs)

        def emit_mains(b):
            st = stage[b]
            xt, istdb, auxrhs = st["xt"], st["istdb"], st["auxrhs"]
            ot = osb.tile([128, NET, TB], F16)
            bsegs = segs[b]
            for j0 in range(0, NET, 4):
                ops = []
                for j in range(j0, j0 + 4):
                    op = opsum.tile([128, TB], F32)
                    ops.append(op)
                    esl = slice(j * 128, (j + 1) * 128)
                    first = True
                    for (c, t0, n) in bsegs:
                        tsl = slice(t0, t0 + n)
                        for k in range(NKC):
                            nc.tensor.matmul(
                                op[:, tsl], w2c[:, c, k, esl], xt[:, k, tsl],
                                start=first, stop=False)
                            first = False
                # aux burst: 4 row groups run concurrently on the PE array
                for j in range(j0, j0 + 4):
                    op = ops[j - j0]
                    esl = slice(j * 128, (j + 1) * 128)
                    r = 32 * (j % 3)   # base partition must be in {0,32,64}
                    for si, (c, t0, n) in enumerate(bsegs):
                        tsl = slice(t0, t0 + n)
                        nc.tensor.matmul(
                            op[:, tsl],
                            gwasb[r : r + 2, c, esl],
                            auxrhs[r : r + 2, tsl],
                            start=False, stop=(si == len(bsegs) - 1))
                for j in range(j0, j0 + 4):
                    nc.vector.tensor_mul(ot[:, j, :], ops[j - j0], istdb)
            nc.sync.dma_start(out=out[b], in_=ot)

        emit_stats(0)
        for b in range(NBLK):
            if b + 1 < NBLK:
                emit_stats(b + 1)
            emit_mains(b)

    nc.compile()
    return nc


_CACHE = {}


def _get_nc(segs, repeat=1):
    key = (segs, repeat)
    if key not in _CACHE:
        _CACHE[key] = _build_nc(segs, repeat)
    return _CACHE[key]


def _run(inputs, repeat=1, emulate=False):
    in_maps, aux = _prep_host(**inputs)
    if emulate:
        results = [_emulate_core(m, aux["segs"]) for m in in_maps]
        return _assemble(results, aux), None
    nc = _get_nc(aux["segs"], repeat)
    res = bass_utils.run_bass_kernel_spmd(
        nc, in_maps, list(range(NCORES)))
    return _assemble(res.results, aux), res


def kernel(**inputs):
    out, _ = _run(inputs)
    return out.astype(np.float32)
